# revision 10
# baseline (speedup 1.0000x reference)
"""SuperposedExpert (K TT-factorized FFN paths + holographic routing) on 8 trn2 cores.

Strategy: expert x data parallel. Core c handles path k = c % 4 for token half
c // 4. The TT cores are expanded to dense W1 [1024,4096] / W2 [4096,1024] on
the host (weight-only preprocessing, same category as the transposes/packing we
already do), with the per-path (1 + path_weight[d]) modulation folded into W2.
On-device per core:
  1. Dense bf16 FFN, fully SBUF-resident weights (W1 8MB + W2 8MB):
     hT = gelu(W1^T @ xT), oT = W2'^T @ hT (PSUM fp32 accum, d-outer ffn2 so
     drains stream out progressively).
  2. logits/softmax gating from bf16 tokens (tiny matmuls on PE, emitted after
     ffn1 chunk 0 so the PE ramps on real work first).
  3. Scale by gate[n], ReduceScatter(add) over the 4 cores sharing the token
     half, one RS per 512-token chunk so chunk 0's RS overlaps chunk 1 compute.
Host only reshapes/casts inputs and concatenates/transposes the output pieces.
"""

import numpy as np
import ml_dtypes

import concourse.bass as bass
import concourse.tile as tile
from concourse import bacc, mybir
from concourse.bass import ds, ts
from concourse.bass_utils import run_bass_kernel_spmd

BF16 = mybir.dt.bfloat16
F32 = mybir.dt.float32
AF = mybir.ActivationFunctionType

K = 4
D = 1024            # d_model
DFF = 4096          # d_ff
NTOK = 2048
NCORES = 8
NHALF = NTOK // 2   # tokens per core
NCH = 512           # n-chunk (psum bank = 512 fp32)
NNCH = NHALF // NCH
GROUPS = [[0, 1, 2, 3], [4, 5, 6, 7]]
# per-chunk ReduceScatter splits over the 8 ffn2 d-tiles
SPLITS = [[4, 4], [4, 3, 1]]


def _emit(nc, tc):
    # ---------------- I/O ----------------
    xTp = nc.dram_tensor("xTp", [128, 8, NHALF], BF16, kind="ExternalInput")
    w1p = nc.dram_tensor("w1p", [128, 8, DFF], BF16, kind="ExternalInput")
    w2p = nc.dram_tensor("w2p", [128, 32, D], BF16, kind="ExternalInput")
    pbT = nc.dram_tensor("pbT", [D, K], BF16, kind="ExternalInput")
    sel = nc.dram_tensor("sel", [K, 1], F32, kind="ExternalInput")
    ones4 = nc.dram_tensor("ones4", [K, 1], F32, kind="ExternalInput")
    ones1 = nc.dram_tensor("ones1", [1, 128], F32, kind="ExternalInput")
    # opiece rows 0-127: d-slice [128k, 128k+128); rows 128-255: [512+128k, ...)
    opiece = nc.dram_tensor("opiece", [D // K, NHALF], BF16, kind="ExternalOutput")

    # RS split per chunk so early collectives overlap ffn2 compute and the
    # last (exposed) one is as small as possible
    cc_in = [[nc.dram_tensor(f"cc_in{i}_{j}", [cnt * 128, NCH], BF16)
              for j, cnt in enumerate(SPLITS[i])] for i in range(NNCH)]
    cc_out = [[nc.dram_tensor(f"cc_out{i}_{j}", [cnt * 32, NCH], BF16)
               for j, cnt in enumerate(SPLITS[i])] for i in range(NNCH)]

    with (
        tc.tile_pool(name="big", bufs=1) as big,
        tc.tile_pool(name="small", bufs=1) as small,
        tc.tile_pool(name="obp", bufs=3) as obp,
        tc.tile_pool(name="pp", bufs=8, space="PSUM") as pp,
    ):
        # ---------------- loads (chunked so compute starts early) ----------
        xt_sb = big.tile([128, 8, NHALF], BF16, tag="xt")
        w1_sb = big.tile([128, 8, DFF], BF16, tag="w1")
        w2_sb = big.tile([128, 32, D], BF16, tag="w2")
        for nh in range(NNCH):
            for t in range(8):
                nc.sync.dma_start(xt_sb[:, t, ts(nh, NCH)], xTp[:, t, ts(nh, NCH)])
        # f-quarter-interleaved so ffn1 group 0 only waits for the first 2MB
        for g in range(4):
            for s in range(8):
                nc.scalar.dma_start(
                    w1_sb[:, s, ts(g, 1024)], w1p[:, s, ts(g, 1024)]
                )
        for q in range(8):
            nc.scalar.dma_start(w2_sb[:, ds(4 * q, 4)], w2p[:, ds(4 * q, 4), :])

        pbt_sb = small.tile([128, 8, K], BF16, tag="pbt")
        nc.sync.dma_start(pbt_sb, pbT.ap().rearrange("(t p) k -> p t k", p=128))
        sel_sb = small.tile([K, 1], F32, tag="sel")
        nc.sync.dma_start(sel_sb, sel.ap())
        ones4_sb = small.tile([K, 1], F32, tag="ones4")
        nc.sync.dma_start(ones4_sb, ones4.ap())
        ones1_sb = small.tile([1, 128], F32, tag="ones1")
        nc.sync.dma_start(ones1_sb, ones1.ap())

        expl = small.tile([K, NHALF], F32, tag="expl")
        gk = small.tile([1, NHALF], F32, tag="gk")
        rden = small.tile([1, NHALF], F32, tag="rden")
        gbc_sb = small.tile([128, NHALF], F32, tag="gbc")

        def gating():
            # logits^T [K, n] = pbT^T @ xT, bf16 with fp32 accum; exp -> softmax
            for n2 in range(NNCH):
                lps = pp.tile([K, NCH], F32, tag="ps", name=f"gl_{n2}")
                for kc in range(8):
                    nc.tensor.matmul(
                        lps, pbt_sb[:, kc], xt_sb[:, kc, ts(n2, NCH)],
                        start=(kc == 0), stop=(kc == 7),
                    )
                nc.scalar.activation(expl[:, ts(n2, NCH)], lps, AF.Exp)
            for n2 in range(NNCH):
                den = pp.tile([1, NCH], F32, tag="ps", name=f"gd_{n2}")
                num = pp.tile([1, NCH], F32, tag="ps", name=f"gn_{n2}")
                nc.tensor.matmul(den, ones4_sb, expl[:, ts(n2, NCH)])
                nc.tensor.matmul(num, sel_sb, expl[:, ts(n2, NCH)])
                nc.vector.reciprocal(rden[:, ts(n2, NCH)], den)
                nc.vector.tensor_mul(gk[:, ts(n2, NCH)], num, rden[:, ts(n2, NCH)])
            # broadcast gate row to 128 partitions: gbc = ones1^T @ gk
            for n2 in range(NNCH):
                gps = pp.tile([128, NCH], F32, tag="ps", name=f"gb_{n2}")
                nc.tensor.matmul(gps, ones1_sb, gk[:, ts(n2, NCH)])
                nc.vector.tensor_copy(gbc_sb[:, ts(n2, NCH)], gps)

        # ---------------- main FFN, n-chunk at a time ----------------
        for nch in range(NNCH):
            ht = big.tile([128, 32, NCH], BF16, tag="ht", name=f"ht_{nch}")
            # ffn1: hT[f, n] = gelu(sum_d W1[d, f] xT[d, n]); s-outer so the
            # first matmuls only need the first W1 d-chunk load
            for grp in range(4):
                ps1 = [
                    pp.tile([128, NCH], F32, tag="ps", name=f"f1_{nch}_{grp}_{j}")
                    for j in range(8)
                ]
                for s in range(8):
                    for j in range(8):
                        m = grp * 8 + j
                        nc.tensor.matmul(
                            ps1[j], w1_sb[:, s, ts(m, 128)],
                            xt_sb[:, s, ts(nch, NCH)],
                            start=(s == 0), stop=(s == 7),
                        )
                for j in range(8):
                    nc.scalar.activation(
                        ht[:, grp * 8 + j], ps1[j], AF.Gelu_apprx_tanh
                    )

            if nch == 0:
                gating()

            # ffn2: oT[d, n] = sum_f W2'[f, d] hT[f, n]; d-outer so each
            # d-tile drains (and its cc_in store issues) as soon as it's done
            m2 = 0
            for j, cnt in enumerate(SPLITS[nch]):
                for i in range(cnt):
                    ps2 = pp.tile([128, NCH], F32, tag="ps", name=f"f2_{nch}_{m2}")
                    for kc in range(32):
                        nc.tensor.matmul(
                            ps2, w2_sb[:, kc, ts(m2, 128)], ht[:, kc],
                            start=(kc == 0), stop=(kc == 31),
                        )
                    ob = obp.tile([128, NCH], BF16, tag="ob", name=f"ob_{nch}_{m2}")
                    nc.vector.tensor_mul(ob, ps2, gbc_sb[:, ts(nch, NCH)])
                    nc.sync.dma_start(cc_in[nch][j][ts(i, 128), :], ob)
                    m2 += 1
                # combine paths for this d-group (overlaps further compute)
                nc.gpsimd.collective_compute(
                    "ReduceScatter",
                    mybir.AluOpType.add,
                    replica_groups=GROUPS,
                    ins=[cc_in[nch][j][:]],
                    outs=[cc_out[nch][j][:]],
                )
                nc.sync.dma_start(
                    opiece[ds(32 * (m2 - cnt), 32 * cnt), ts(nch, NCH)],
                    cc_out[nch][j][:],
                )


def build(verbose=False):
    nc = bacc.Bacc("TRN2", target_bir_lowering=False, debug=False, num_devices=NCORES)
    with tile.TileContext(nc) as tc:
        _emit(nc, tc)
    nc.compile()
    return nc


def _expand_tt(core1, core2, din, dout):
    """Dense W[(a b), (x y)] = sum_r core1[a, x, r] core2[r, b, y]."""
    a, x, r = core1.shape
    r2, b, y = core2.shape
    m = core1.reshape(a * x, r).astype(np.float32) @ \
        core2.reshape(r2, b * y).astype(np.float32)
    w = m.reshape(a, x, b, y).transpose(0, 2, 1, 3).reshape(a * b, x * y)
    assert w.shape == (din, dout)
    return w


def make_in_maps(inputs):
    tokens = inputs["tokens"]
    bf = ml_dtypes.bfloat16
    in_maps = []
    w1_cache, w2_cache = {}, {}
    for c in range(NCORES):
        half, k = c // 4, c % 4
        tok = tokens[half * NHALF:(half + 1) * NHALF]
        xt = np.ascontiguousarray(
            tok.T.reshape(8, 128, NHALF).transpose(1, 0, 2)
        ).astype(bf)
        if k not in w1_cache:
            w1 = _expand_tt(inputs["ffn1_core1"][k], inputs["ffn1_core2"][k],
                            D, DFF)
            w1_cache[k] = np.ascontiguousarray(
                w1.reshape(8, 128, DFF).transpose(1, 0, 2)
            ).astype(bf)
            w2 = _expand_tt(inputs["ffn2_core1"][k], inputs["ffn2_core2"][k],
                            DFF, D)
            w2 *= (1.0 + inputs["path_weights"][k])[None, :]
            w2_cache[k] = np.ascontiguousarray(
                w2.reshape(32, 128, D).transpose(1, 0, 2)
            ).astype(bf)
        pbt = np.ascontiguousarray(inputs["path_bases"].T).astype(bf)
        selk = np.zeros((K, 1), np.float32)
        selk[k, 0] = 1.0
        in_maps.append({
            "xTp": xt,
            "w1p": w1_cache[k], "w2p": w2_cache[k],
            "pbT": pbt, "sel": selk,
            "ones4": np.ones((K, 1), np.float32),
            "ones1": np.ones((1, 128), np.float32),
        })
    return in_maps


def assemble(results):
    out = np.empty((NTOK, D), np.float32)
    for c in range(NCORES):
        half, k = c // 4, c % 4
        piece = results[c]["opiece"].astype(np.float32)  # [256 d-shards, 1024 n]
        for nch in range(NNCH):
            cols = slice(nch * NCH, (nch + 1) * NCH)
            rows = slice(half * NHALF + nch * NCH,
                         half * NHALF + (nch + 1) * NCH)
            start = 0
            for cnt in SPLITS[nch]:
                # shard j covers d in [128*start + k*32*cnt, ... + 32*cnt)
                dlo = 128 * start + k * 32 * cnt
                out[rows, dlo:dlo + 32 * cnt] = \
                    piece[32 * start:32 * start + 32 * cnt, cols].T
                start += cnt
    return out


_NC = None


def run(inputs, trace=False):
    global _NC
    if _NC is None:
        _NC = build()
    res = run_bass_kernel_spmd(
        _NC, make_in_maps(inputs), core_ids=list(range(NCORES)), trace=trace
    )
    return assemble(res.results), res


def kernel(**inputs):
    out, _ = run(inputs)
    return out


# revision 12
# speedup vs baseline: 1.2158x; 1.2158x over previous
"""SuperposedExpert (K TT-factorized FFN paths + holographic routing) on 8 trn2 cores.

Strategy: expert x data parallel. Core c handles path k = c % 4 for token half
c // 4. The TT cores are expanded to dense W1 [1024,4096] / W2 [4096,1024] on
the host (weight-only preprocessing, same category as the transposes/packing we
already do), with the per-path (1 + path_weight[d]) modulation folded into W2.
On-device per core:
  1. Dense bf16 FFN, fully SBUF-resident weights (W1 8MB + W2 8MB):
     hT = gelu(W1^T @ xT), oT = W2'^T @ hT (PSUM fp32 accum, d-outer ffn2 so
     drains stream out progressively).
  2. logits/softmax gating from bf16 tokens (tiny matmuls on PE, emitted after
     ffn1 chunk 0 so the PE ramps on real work first).
  3. Scale by gate[n], ReduceScatter(add) over the 4 cores sharing the token
     half, one RS per 512-token chunk so chunk 0's RS overlaps chunk 1 compute.
Host only reshapes/casts inputs and concatenates/transposes the output pieces.
"""

import numpy as np
import ml_dtypes

import concourse.bass as bass
import concourse.tile as tile
from concourse import bacc, mybir
from concourse.bass import ds, ts
from concourse.bass_utils import run_bass_kernel_spmd

BF16 = mybir.dt.bfloat16
F32 = mybir.dt.float32
AF = mybir.ActivationFunctionType

K = 4
D = 1024            # d_model
DFF = 4096          # d_ff
NTOK = 2048
NCORES = 8
NHALF = NTOK // 2   # tokens per core
NCH = 512           # n-chunk (psum bank = 512 fp32)
NNCH = NHALF // NCH
GROUPS = [[0, 1, 2, 3], [4, 5, 6, 7]]
# per-chunk ReduceScatter splits over the 8 ffn2 d-tiles
SPLITS = [[4, 4], [4, 4]]


def _emit(nc, tc):
    # ---------------- I/O ----------------
    xTp = nc.dram_tensor("xTp", [128, 8, NHALF], BF16, kind="ExternalInput")
    w1p = nc.dram_tensor("w1p", [128, 8, DFF], BF16, kind="ExternalInput")
    w2p = nc.dram_tensor("w2p", [128, 32, D], BF16, kind="ExternalInput")
    pbT = nc.dram_tensor("pbT", [D, K], BF16, kind="ExternalInput")
    sel = nc.dram_tensor("sel", [K, 1], F32, kind="ExternalInput")
    ones4 = nc.dram_tensor("ones4", [K, 1], F32, kind="ExternalInput")
    ones1 = nc.dram_tensor("ones1", [1, 128], F32, kind="ExternalInput")
    # opiece rows 0-127: d-slice [128k, 128k+128); rows 128-255: [512+128k, ...)
    opiece = nc.dram_tensor("opiece", [D // K, NHALF], BF16, kind="ExternalOutput")

    # RS split per chunk so early collectives overlap ffn2 compute and the
    # last (exposed) one is as small as possible
    cc_in = [[nc.dram_tensor(f"cc_in{i}_{j}", [cnt * 128, NCH], BF16)
              for j, cnt in enumerate(SPLITS[i])] for i in range(NNCH)]
    cc_out = [[nc.dram_tensor(f"cc_out{i}_{j}", [cnt * 32, NCH], BF16)
               for j, cnt in enumerate(SPLITS[i])] for i in range(NNCH)]

    with (
        tc.tile_pool(name="big", bufs=1) as big,
        tc.tile_pool(name="small", bufs=1) as small,
        tc.tile_pool(name="obp", bufs=3) as obp,
        tc.tile_pool(name="pp", bufs=8, space="PSUM") as pp,
    ):
        # ---------------- loads (chunked so compute starts early) ----------
        xt_sb = big.tile([128, 8, NHALF], BF16, tag="xt")
        w1_sb = big.tile([128, 8, DFF], BF16, tag="w1")
        w2_sb = big.tile([128, 32, D], BF16, tag="w2")
        for nh in range(NNCH):
            for t in range(8):
                nc.sync.dma_start(xt_sb[:, t, ts(nh, NCH)], xTp[:, t, ts(nh, NCH)])
        for s in range(8):
            nc.scalar.dma_start(w1_sb[:, s], w1p[:, s, :])
        for q in range(8):
            nc.scalar.dma_start(w2_sb[:, ds(4 * q, 4)], w2p[:, ds(4 * q, 4), :])

        pbt_sb = small.tile([128, 8, K], BF16, tag="pbt")
        nc.sync.dma_start(pbt_sb, pbT.ap().rearrange("(t p) k -> p t k", p=128))
        sel_sb = small.tile([K, 1], F32, tag="sel")
        nc.sync.dma_start(sel_sb, sel.ap())
        ones4_sb = small.tile([K, 1], F32, tag="ones4")
        nc.sync.dma_start(ones4_sb, ones4.ap())
        ones1_sb = small.tile([1, 128], F32, tag="ones1")
        nc.sync.dma_start(ones1_sb, ones1.ap())

        expl = small.tile([K, NHALF], F32, tag="expl")
        gk = small.tile([1, NHALF], F32, tag="gk")
        rden = small.tile([1, NHALF], F32, tag="rden")
        gbc_sb = small.tile([128, NHALF], F32, tag="gbc")

        def gating():
            # logits^T [K, n] = pbT^T @ xT, bf16 with fp32 accum; exp -> softmax
            for n2 in range(NNCH):
                lps = pp.tile([K, NCH], F32, tag="ps", name=f"gl_{n2}")
                for kc in range(8):
                    nc.tensor.matmul(
                        lps, pbt_sb[:, kc], xt_sb[:, kc, ts(n2, NCH)],
                        start=(kc == 0), stop=(kc == 7),
                    )
                nc.scalar.activation(expl[:, ts(n2, NCH)], lps, AF.Exp)
            for n2 in range(NNCH):
                den = pp.tile([1, NCH], F32, tag="ps", name=f"gd_{n2}")
                num = pp.tile([1, NCH], F32, tag="ps", name=f"gn_{n2}")
                nc.tensor.matmul(den, ones4_sb, expl[:, ts(n2, NCH)])
                nc.tensor.matmul(num, sel_sb, expl[:, ts(n2, NCH)])
                nc.vector.reciprocal(rden[:, ts(n2, NCH)], den)
                nc.vector.tensor_mul(gk[:, ts(n2, NCH)], num, rden[:, ts(n2, NCH)])
            # broadcast gate row to 128 partitions: gbc = ones1^T @ gk
            for n2 in range(NNCH):
                gps = pp.tile([128, NCH], F32, tag="ps", name=f"gb_{n2}")
                nc.tensor.matmul(gps, ones1_sb, gk[:, ts(n2, NCH)])
                nc.vector.tensor_copy(gbc_sb[:, ts(n2, NCH)], gps)

        # ---------------- main FFN, n-chunk at a time ----------------
        for nch in range(NNCH):
            ht = big.tile([128, 32, NCH], BF16, tag="ht", name=f"ht_{nch}")
            # ffn1: hT[f, n] = gelu(sum_d W1[d, f] xT[d, n]); s-outer so the
            # first matmuls only need the first W1 d-chunk load
            for grp in range(4):
                ps1 = [
                    pp.tile([128, NCH], F32, tag="ps", name=f"f1_{nch}_{grp}_{j}")
                    for j in range(8)
                ]
                for s in range(8):
                    for j in range(8):
                        m = grp * 8 + j
                        nc.tensor.matmul(
                            ps1[j], w1_sb[:, s, ts(m, 128)],
                            xt_sb[:, s, ts(nch, NCH)],
                            start=(s == 0), stop=(s == 7),
                        )
                for j in range(8):
                    nc.scalar.activation(
                        ht[:, grp * 8 + j], ps1[j], AF.Gelu_apprx_tanh
                    )

            if nch == 0:
                gating()

            # ffn2: oT[d, n] = sum_f W2'[f, d] hT[f, n]; d-outer so each
            # d-tile drains (and its cc_in store issues) as soon as it's done
            m2 = 0
            for j, cnt in enumerate(SPLITS[nch]):
                for i in range(cnt):
                    ps2 = pp.tile([128, NCH], F32, tag="ps", name=f"f2_{nch}_{m2}")
                    for kc in range(32):
                        nc.tensor.matmul(
                            ps2, w2_sb[:, kc, ts(m2, 128)], ht[:, kc],
                            start=(kc == 0), stop=(kc == 31),
                        )
                    ob = obp.tile([128, NCH], BF16, tag="ob", name=f"ob_{nch}_{m2}")
                    nc.vector.tensor_mul(ob, ps2, gbc_sb[:, ts(nch, NCH)])
                    nc.sync.dma_start(cc_in[nch][j][ts(i, 128), :], ob)
                    m2 += 1
                # combine paths for this d-group (overlaps further compute)
                nc.gpsimd.collective_compute(
                    "ReduceScatter",
                    mybir.AluOpType.add,
                    replica_groups=GROUPS,
                    ins=[cc_in[nch][j][:]],
                    outs=[cc_out[nch][j][:]],
                )
                nc.sync.dma_start(
                    opiece[ds(32 * (m2 - cnt), 32 * cnt), ts(nch, NCH)],
                    cc_out[nch][j][:],
                )


def build(verbose=False):
    nc = bacc.Bacc("TRN2", target_bir_lowering=False, debug=False, num_devices=NCORES)
    with tile.TileContext(nc) as tc:
        _emit(nc, tc)
    nc.compile()
    return nc


def _expand_tt(core1, core2, din, dout):
    """Dense W[(a b), (x y)] = sum_r core1[a, x, r] core2[r, b, y]."""
    a, x, r = core1.shape
    r2, b, y = core2.shape
    m = core1.reshape(a * x, r).astype(np.float32) @ \
        core2.reshape(r2, b * y).astype(np.float32)
    w = m.reshape(a, x, b, y).transpose(0, 2, 1, 3).reshape(a * b, x * y)
    assert w.shape == (din, dout)
    return w


def make_in_maps(inputs):
    tokens = inputs["tokens"]
    bf = ml_dtypes.bfloat16
    in_maps = []
    w1_cache, w2_cache = {}, {}
    for c in range(NCORES):
        half, k = c // 4, c % 4
        tok = tokens[half * NHALF:(half + 1) * NHALF]
        xt = np.ascontiguousarray(
            tok.T.reshape(8, 128, NHALF).transpose(1, 0, 2)
        ).astype(bf)
        if k not in w1_cache:
            w1 = _expand_tt(inputs["ffn1_core1"][k], inputs["ffn1_core2"][k],
                            D, DFF)
            w1_cache[k] = np.ascontiguousarray(
                w1.reshape(8, 128, DFF).transpose(1, 0, 2)
            ).astype(bf)
            w2 = _expand_tt(inputs["ffn2_core1"][k], inputs["ffn2_core2"][k],
                            DFF, D)
            w2 *= (1.0 + inputs["path_weights"][k])[None, :]
            w2_cache[k] = np.ascontiguousarray(
                w2.reshape(32, 128, D).transpose(1, 0, 2)
            ).astype(bf)
        pbt = np.ascontiguousarray(inputs["path_bases"].T).astype(bf)
        selk = np.zeros((K, 1), np.float32)
        selk[k, 0] = 1.0
        in_maps.append({
            "xTp": xt,
            "w1p": w1_cache[k], "w2p": w2_cache[k],
            "pbT": pbt, "sel": selk,
            "ones4": np.ones((K, 1), np.float32),
            "ones1": np.ones((1, 128), np.float32),
        })
    return in_maps


def assemble(results):
    out = np.empty((NTOK, D), np.float32)
    for c in range(NCORES):
        half, k = c // 4, c % 4
        piece = results[c]["opiece"].astype(np.float32)  # [256 d-shards, 1024 n]
        for nch in range(NNCH):
            cols = slice(nch * NCH, (nch + 1) * NCH)
            rows = slice(half * NHALF + nch * NCH,
                         half * NHALF + (nch + 1) * NCH)
            start = 0
            for cnt in SPLITS[nch]:
                # shard j covers d in [128*start + k*32*cnt, ... + 32*cnt)
                dlo = 128 * start + k * 32 * cnt
                out[rows, dlo:dlo + 32 * cnt] = \
                    piece[32 * start:32 * start + 32 * cnt, cols].T
                start += cnt
    return out


_NC = None


def run(inputs, trace=False):
    global _NC
    if _NC is None:
        _NC = build()
    res = run_bass_kernel_spmd(
        _NC, make_in_maps(inputs), core_ids=list(range(NCORES)), trace=trace
    )
    return assemble(res.results), res


def kernel(**inputs):
    out, _ = run(inputs)
    return out
